# revision 15
# baseline (speedup 1.0000x reference)
"""Bezier-to-image Gaussian splat kernel for Trainium2 (8 NeuronCores).

Reference computation (per sample b of 256):
    T = warped cubic Bernstein basis (30, 4)
    points = einsum('nk,blkc->blnc', T, x.reshape(B,160,4,2))   # (B,160,30,2)
    gx[b,l,i,n] = exp(-(i/60 - X[b,l,n])^2 / 2e-4)
    out[b,i,j]  = min(sum_{l,n} gx[b,l,i,n]*gy[b,l,j,n], 1)     # (B,60,60)

Strategy: pure data parallel, 32 samples per core.  Per sample the 4800
bezier points sit on 128 partitions (4 curves x 32-row strips, rows 30/31
of each strip dead); each point needs a 60-cell Gaussian band per axis.

The two axes take different paths so no single engine owns both:
  x-side: dd_x = iota - rx on DVE (broadcast tensor_tensor, 1x, 2400 el),
          then ScalarE Derivative_Erf -> gx (fp16, SBUF).
  y-side: a persistent 5-bank PSUM tile holds (60*Y_b - j) directly.  It is
          initialised once with the -j iota pattern, and per sample the PE
          accumulates delta-r = 60*(Y_b - Y_{b-1}) via matmuls whose moving
          operand broadcasts each control-point column over the 60 cells
          (2400 cols/sample ~ 1us on PE).  ScalarE reads the PSUM bands
          directly -> gy.
The 60x60 image accumulates on PE as sum_c Gx_c^T @ Gy_c in one PSUM bank.
"""

import math
import os

import numpy as np
import orjson

import bass_rust
import concourse.bass as bass
import concourse.mybir as mybir
import concourse.tile as tile
from concourse.bass_utils import run_bass_kernel_spmd

B, L, N, W = 256, 160, 30, 60
NCORES = 8
BC = B // NCORES          # samples per core
ALPHA = 2e-4
KEXP = 1.0 / (W * W * ALPHA)          # exponent scale in cell units: 1/0.72
SDERF = math.sqrt(KEXP)               # Derivative_Erf input scale
DERF_FIX = math.pi / 4.0              # undo (2/sqrt(pi))^2 from Derivative_Erf
CHUNKS = 40                           # 4 curves x 30 samples per chunk
PTS = 128                             # chunk partition dim: p = 32*lg + n
CW = 60                               # band width (= W)
NBANK = 5                             # PSUM banks holding the y-side bands
BANKC = CHUNKS // NBANK               # chunks per PSUM bank (8 -> 480 cols)

LAST_RESULTS = None  # test harness reads profiling info from here


def _basis_T() -> np.ndarray:
    t = np.arange(N, dtype=np.float32) / np.float32(N)
    t = 2 * t**3 - 3 * t**2 + 2 * t
    t_3_0 = t**3
    t_2_1 = t**2 - t_3_0
    t_1_2 = t_3_0 - 2 * t**2 + t
    t_0_3 = (1 - t) ** 3
    return np.stack([t_3_0, 3 * t_2_1, 3 * t_1_2, t_0_3], axis=1).astype(np.float32)


def _legalize_waits(nc, max_waits: int = 1):
    """Walrus rejects engine instructions carrying more than ~1 sync wait
    ("Too many sync wait commands").  Hoist excess waits onto same-engine
    Drain instructions inserted immediately before the offender."""
    js = orjson.loads(mybir.module_to_json_bytes(nc.m))
    ctr = 0
    for f in js["functions"]:
        for bb in f["blocks"]:
            out = []
            changed = False
            for inst in bb["instructions"]:
                si = inst.get("sync_info")
                waits = si.get("on_wait") if si else None
                if waits and len(waits) > max_waits:
                    keep = waits[:max_waits]
                    for w in waits[max_waits:]:
                        ctr += 1
                        out.append({
                            "debug": inst.get("debug", 0),
                            "engine": inst["engine"],
                            "ins": [], "outs": [],
                            "name": f"waitfix-{ctr}",
                            "opcode": "Drain",
                            "sync_info": {"on_update": [], "on_wait": [w]},
                        })
                    si["on_wait"] = keep
                    changed = True
                out.append(inst)
            if changed:
                bb["instructions"] = out
    if ctr:
        nc.m = bass_rust.module_from_json_bytes(orjson.dumps(js))
    return ctr


def build_program(legalize: bool = True):
    f32 = mybir.dt.float32
    f16 = mybir.dt.float16

    nc = bass.Bass("TRN2", target_bir_lowering=False, debug=False)

    x_t = nc.dram_tensor("x", [BC, L, 8], f32, kind="ExternalInput")
    y_t = nc.dram_tensor("y", [BC, W, W], f32, kind="ExternalOutput")

    # (4, 32) stationary operand for the x-side r matmul: r[m] = 60*X.
    tsc_np = np.zeros((4, 32), dtype=np.float32)
    tsc_np[:, :N] = (W * _basis_T()).T
    tsc_d = nc.inline_tensor(tsc_np, name="tscT")

    # (16, 128) stationary operand for the y-side delta matmuls:
    # tscB[(k,lg), 32*lg'+n] = 60*T[n,k] * [lg==lg']
    tscB_np = np.zeros((16, 128), dtype=np.float32)
    for lg in range(4):
        tscB_np[4 * lg : 4 * lg + 4, 32 * lg : 32 * lg + N] = (W * _basis_T()).T
    tscB_d = nc.inline_tensor(tscB_np, name="tscB")

    # x-side iota: dead rows (n in {30,31} of each strip) get +120 so their
    # band distance is >= 60 -> gx = 0, killing the dead rows' contribution
    # to the image outer product regardless of the (unmasked) y side.
    iota_np = np.tile(np.arange(CW, dtype=np.float16), (PTS, 1))
    for lg in range(4):
        iota_np[32 * lg + 30 : 32 * lg + 32, :] += np.float16(120.0)
    iota_d = nc.inline_tensor(iota_np, name="iota60")

    # ones column for the y-side -j prefill (lhsT = [1, 128])
    ones_np = np.ones((1, PTS), dtype=np.float32)
    ones_d = nc.inline_tensor(ones_np, name="ones1")
    # -j pattern, repeated per chunk: [1, 2400]
    negj_np = -np.tile(np.arange(CW, dtype=np.float32), CHUNKS)[None, :]
    negj_d = nc.inline_tensor(negj_np, name="negj")

    with tile.TileContext(nc) as tc, tc.tile_pool(name="const", bufs=1) as cpool, \
            tc.tile_pool(name="ctrl", bufs=1) as ctrl_pool, \
            tc.tile_pool(name="outp", bufs=1) as out_pool, \
            tc.tile_pool(name="stage", bufs=4) as stage_pool, \
            tc.tile_pool(name="dwork", bufs=3) as dpool, \
            tc.tile_pool(name="band", bufs=4) as band_pool, \
            tc.tile_pool(name="ypsum", bufs=1, space="PSUM") as yps_pool, \
            tc.tile_pool(name="rpsum", bufs=2, space="PSUM") as rps_pool, \
            tc.tile_pool(name="imgpsum", bufs=1, space="PSUM") as img_pool:

        # Prologue: DMA loads land in staging tiles; DVE copies them into the
        # tiles PE reads (PE LDWEIGHTS tolerates very few sync waits).
        tsc0 = cpool.tile([4, 32], f32, tag="tsc0")
        nc.sync.dma_start(tsc0[:], tsc_d.ap())
        tsc = cpool.tile([4, 32], f32, tag="tsc")
        nc.vector.tensor_copy(tsc[:], tsc0[:])
        tscB0 = cpool.tile([16, PTS], f32, tag="tscB0")
        nc.sync.dma_start(tscB0[:], tscB_d.ap())
        tscB = cpool.tile([16, PTS], f32, tag="tscB")
        nc.vector.tensor_copy(tscB[:], tscB0[:])
        ones0 = cpool.tile([1, PTS], f32, tag="ones0")
        nc.sync.dma_start(ones0[:], ones_d.ap())
        ones1 = cpool.tile([1, PTS], f32, tag="ones1")
        nc.vector.tensor_copy(ones1[:], ones0[:])
        negj0 = cpool.tile([1, CHUNKS * CW], f32, tag="negj0")
        nc.sync.dma_start(negj0[:], negj_d.ap())
        negj = cpool.tile([1, CHUNKS * CW], f32, tag="negj")
        nc.vector.tensor_copy(negj[:], negj0[:])
        iot = cpool.tile([PTS, CW], f16, tag="iota")
        nc.sync.dma_start(iot[:], iota_d.ap())

        # control points, two layouts, loaded in groups of 8 samples:
        #   ctA [4=k, (b, l, c)]        for the x-side r matmuls
        #   ctB [16=(k,lg), (b, c40)]   y coords, for the delta matmuls
        GRP = 8
        NGRP = BC // GRP
        ct = ctrl_pool.tile([4, BC * 2 * L], f32, tag="ct")
        ctB = ctrl_pool.tile([16, BC * CHUNKS], f32, tag="ctB")
        gsz = GRP * 2 * L
        ctA_stage, ctB_stage = [], []
        for g in range(NGRP):
            ct0 = stage_pool.tile([4, gsz], f32, tag="ct0")
            nc.sync.dma_start(
                ct0[:].rearrange("k (b l c) -> k b l c", b=GRP, c=2),
                x_t.ap()[g * GRP : (g + 1) * GRP]
                .rearrange("b l (k c) -> k b l c", k=4),
            )
            ctA_stage.append(ct0)
            # partition q = 4*lg + k: merges with k's stride-2 into one
            # stride-2 run of 16, so the DMA AP balances as plain 2D.
            cb0 = stage_pool.tile([16, GRP * CHUNKS], f32, tag="cb0")
            nc.sync.dma_start(
                cb0[:].rearrange("q (bc o) -> q bc o", o=1),
                x_t.ap()[g * GRP : (g + 1) * GRP]
                .rearrange("b (c lg) (k two) -> (lg k) (b c) two", lg=4, k=4)[
                    :, :, 1:2
                ],
            )
            ctB_stage.append(cb0)
        ct_v = ct[:].rearrange("k (b c g co) -> k b c g co", b=BC, c=CHUNKS, co=2)

        # all 32 output images live here until the per-group DMAs
        out_all = out_pool.tile([W, BC * W], f32, tag="oall")

        # persistent y-side band accumulator, fp32, 5 PSUM banks.  Each bank
        # holds 8 chunks x 60 cells in its first 480 fp32 slots (a matmul
        # write may not cross a bank boundary, so banks are padded to 512).
        # value = -j + 60*Y(current sample)
        BANKW = 512
        ddy = yps_pool.tile([PTS, NBANK * BANKW], f32, tag="ddy")
        ddy_act = ddy[:].rearrange("p (kb w) -> p kb w", kb=NBANK)[
            :, :, 0 : BANKC * CW
        ]
        for kb in range(NBANK):
            nc.tensor.matmul(
                ddy[:, kb * BANKW : kb * BANKW + BANKC * CW],
                lhsT=ones1[:],
                rhs=negj[:, kb * BANKC * CW : (kb + 1) * BANKC * CW],
                start=True,
                stop=False,
                skip_group_check=True,
            )

        groups_emitted = [False] * NGRP
        r_tiles = {}
        dy_tiles = {}

        def emit_pre(b):
            """ct copies + x-side r matmul + y-side delta operand for b."""
            g = b // GRP
            if not groups_emitted[g]:
                groups_emitted[g] = True
                nc.vector.tensor_copy(
                    ct[:, g * gsz : (g + 1) * gsz], ctA_stage[g][:]
                )
                nc.vector.tensor_copy(
                    ctB[:, g * GRP * CHUNKS : (g + 1) * GRP * CHUNKS],
                    ctB_stage[g][:],
                )
            r_ps = rps_pool.tile([PTS, CHUNKS], f32, tag="rps")
            for lg in range(4):
                nc.tensor.matmul(
                    r_ps[32 * lg : 32 * lg + 32, :],
                    lhsT=tsc[:],
                    rhs=ct_v[:, b : b + 1, :, lg : lg + 1, 0:1],
                    start=True,
                    stop=True,
                    tile_position=(0, 32 * lg),
                )
            r_tiles[b] = r_ps
            # delta-Y operand: ctB[b] - ctB[b-1]  (b=0: ctB[0] applied as-is)
            dy = dpool.tile([16, CHUNKS], f32, tag="dy")
            if b == 0:
                nc.vector.tensor_copy(dy[:], ctB[:, 0:CHUNKS])
            else:
                nc.vector.tensor_tensor(
                    dy[:],
                    ctB[:, b * CHUNKS : (b + 1) * CHUNKS],
                    ctB[:, (b - 1) * CHUNKS : b * CHUNKS],
                    mybir.AluOpType.subtract,
                )
            dy_tiles[b] = dy

        emit_pre(0)
        emit_pre(1)

        for b in range(BC):
            if b + 2 < BC:
                emit_pre(b + 2)
            r_ps = r_tiles.pop(b)

            # ---- y-side: accumulate 60*(Y_b - Y_{b-1}) into the PSUM bands
            dy = dy_tiles.pop(b)
            dy_bc = dy[:].rearrange("q (c o) -> q c o", o=1).broadcast_to(
                [16, CHUNKS, CW]
            )
            for kb in range(NBANK):
                nc.tensor.matmul(
                    ddy[:, kb * BANKW : kb * BANKW + BANKC * CW]
                    .rearrange("p (c w) -> p c w", w=CW),
                    lhsT=tscB[:],
                    rhs=dy_bc[:, kb * BANKC : (kb + 1) * BANKC],
                    start=False,
                    stop=(b == BC - 1),
                    skip_group_check=True,
                )

            # ---- x-side distance on DVE (iota broadcast over chunks) ----
            ddx = band_pool.tile([PTS, CHUNKS * CW], f16, tag="ddx")
            nc.vector.tensor_tensor(
                ddx[:].rearrange("p (c w) -> p c w", w=CW),
                iot[:].rearrange("p (o w) -> p o w", o=1).broadcast_to(
                    [PTS, CHUNKS, CW]
                ),
                r_ps[:].rearrange("p (c o) -> p c o", o=1).broadcast_to(
                    [PTS, CHUNKS, CW]
                ),
                mybir.AluOpType.subtract,
            )

            # ---- Gaussians: DErf = 2/sqrt(pi) * exp(-x^2) ----
            ggx = band_pool.tile([PTS, CHUNKS * CW], f16, tag="ggx")
            nc.scalar.activation(
                ggx[:], ddx[:],
                mybir.ActivationFunctionType.Derivative_Erf,
                bias=0.0, scale=SDERF,
            )
            ggy = band_pool.tile([PTS, CHUNKS * CW], f16, tag="ggy")
            nc.scalar.activation(
                ggy[:].rearrange("p (kb cw) -> p kb cw", kb=NBANK),
                ddy_act,
                mybir.ActivationFunctionType.Derivative_Erf,
                bias=0.0, scale=SDERF,
            )

            # ---- image accumulation: sum_c Gx_c^T @ Gy_c ----
            img = img_pool.tile([W, W], f32, tag="img")
            for c in range(CHUNKS):
                nc.tensor.matmul(
                    img[:],
                    lhsT=ggx[:, CW * c : CW * c + W],
                    rhs=ggy[:, CW * c : CW * c + W],
                    start=(c == 0),
                    stop=(c == CHUNKS - 1),
                )

            # ---- min(scale*img, 1) -> staging ----
            nc.vector.tensor_scalar(
                out_all[:, W * b : W * (b + 1)],
                img[:],
                DERF_FIX,
                1.0,
                mybir.AluOpType.mult,
                mybir.AluOpType.min,
            )

            if b % GRP == GRP - 1:
                g = b // GRP
                nc.sync.dma_start(
                    y_t.ap()[g * GRP : (g + 1) * GRP].rearrange("b i j -> i b j"),
                    out_all[:, W * GRP * g : W * GRP * (g + 1)]
                    .rearrange("i (b j) -> i b j", b=GRP),
                )

    if legalize:
        _legalize_waits(nc)
    return nc


_PROGRAM = None


def kernel(x: np.ndarray, _trace: bool = False) -> np.ndarray:
    global _PROGRAM, LAST_RESULTS
    assert x.shape == (B, L, 8) and x.dtype == np.float32, (x.shape, x.dtype)
    if _PROGRAM is None:
        _PROGRAM = build_program()
    nc = _PROGRAM
    shards = np.split(np.ascontiguousarray(x), NCORES, axis=0)
    in_maps = [{"x": s} for s in shards]
    res = run_bass_kernel_spmd(nc, in_maps, list(range(NCORES)), trace=_trace)
    LAST_RESULTS = res
    return np.concatenate([res.results[i]["y"] for i in range(NCORES)], axis=0)


# revision 16
# speedup vs baseline: 2.0901x; 2.0901x over previous
"""Bezier-to-image Gaussian splat kernel for Trainium2 (8 NeuronCores).

Reference computation (per sample b of 256):
    T = warped cubic Bernstein basis (30, 4)
    points = einsum('nk,blkc->blnc', T, x.reshape(B,160,4,2))   # (B,160,30,2)
    gx[b,l,i,n] = exp(-(i/60 - X[b,l,n])^2 / 2e-4)
    out[b,i,j]  = min(sum_{l,n} gx[b,l,i,n]*gy[b,l,j,n], 1)     # (B,60,60)

Strategy: pure data parallel, 32 samples per core.  Per sample the 4800
bezier points are processed in 40 chunks of 128 points (4 curves x 32-row
strips; rows 30/31 of each strip are dead and killed via the iota constant);
d[p,i] = i - 60*X_p is built by one broadcast DVE tensor_tensor reading r
straight from PSUM, the Gaussian is evaluated on ScalarE (Derivative_Erf
LUT = 2/sqrt(pi)*exp(-x^2) in a single batched pass), and the 60x60 image
accumulates on PE as sum_c GxT_c^T @ GyT_c in one PSUM bank.

Pipelining: the r matmuls are emitted two samples ahead so the DVE subtract
never waits on PE, and the (PSUM-dependent) min/scale op for sample b is
emitted one iteration late so its wait on the image matmuls overlaps the
next sample's subtract instead of stalling the in-order DVE queue.
"""

import math

import numpy as np
import orjson

import bass_rust
import concourse.bass as bass
import concourse.mybir as mybir
import concourse.tile as tile
from concourse.bass_utils import run_bass_kernel_spmd

B, L, N, W = 256, 160, 30, 60
NCORES = 8
BC = B // NCORES          # samples per core
ALPHA = 2e-4
KEXP = 1.0 / (W * W * ALPHA)          # exponent scale in cell units: 1/0.72
SDERF = math.sqrt(KEXP)               # Derivative_Erf input scale
DERF_FIX = math.pi / 4.0              # undo (2/sqrt(pi))^2 from Derivative_Erf
CHUNKS = 40                           # 4 curves x 30 samples per chunk
PTS = 128                             # chunk partition dim: p = 32*lg + n
CW = 60                               # width of one chunk's band (= W)

LAST_RESULTS = None  # test harness reads profiling info from here


def _basis_T() -> np.ndarray:
    t = np.arange(N, dtype=np.float32) / np.float32(N)
    t = 2 * t**3 - 3 * t**2 + 2 * t
    t_3_0 = t**3
    t_2_1 = t**2 - t_3_0
    t_1_2 = t_3_0 - 2 * t**2 + t
    t_0_3 = (1 - t) ** 3
    return np.stack([t_3_0, 3 * t_2_1, 3 * t_1_2, t_0_3], axis=1).astype(np.float32)


def _legalize_waits(nc, max_waits: int = 1):
    """Walrus rejects engine instructions carrying more than ~1 sync wait
    ("Too many sync wait commands").  Hoist excess waits onto same-engine
    Drain instructions inserted immediately before the offender."""
    js = orjson.loads(mybir.module_to_json_bytes(nc.m))
    ctr = 0
    for f in js["functions"]:
        for bb in f["blocks"]:
            out = []
            changed = False
            for inst in bb["instructions"]:
                si = inst.get("sync_info")
                waits = si.get("on_wait") if si else None
                if waits and len(waits) > max_waits:
                    keep = waits[:max_waits]
                    for w in waits[max_waits:]:
                        ctr += 1
                        out.append({
                            "debug": inst.get("debug", 0),
                            "engine": inst["engine"],
                            "ins": [], "outs": [],
                            "name": f"waitfix-{ctr}",
                            "opcode": "Drain",
                            "sync_info": {"on_update": [], "on_wait": [w]},
                        })
                    si["on_wait"] = keep
                    changed = True
                out.append(inst)
            if changed:
                bb["instructions"] = out
    if ctr:
        nc.m = bass_rust.module_from_json_bytes(orjson.dumps(js))
    return ctr


def build_program(legalize: bool = True):
    f32 = mybir.dt.float32
    f16 = mybir.dt.float16

    nc = bass.Bass("TRN2", target_bir_lowering=False, debug=False)

    x_t = nc.dram_tensor("x", [BC, L, 8], f32, kind="ExternalInput")
    y_t = nc.dram_tensor("y", [BC, W, W], f32, kind="ExternalOutput")

    # (4, 32) stationary operand: r[m] = sum_k TscT[k,m]*ctrl[k] = 60*X.
    tsc_np = np.zeros((4, 32), dtype=np.float32)
    tsc_np[:, :N] = (W * _basis_T()).T
    tsc_d = nc.inline_tensor(tsc_np, name="tscT")

    # iota rows for the dead partitions (n in {30,31} of each 32-strip) are
    # offset +120 so their band distance is >= 60 -> g = 0; no memset or
    # ones-row needed to neutralise them.
    iota_np = np.tile(np.arange(CW, dtype=np.float16), (PTS, 1))
    for lg in range(4):
        iota_np[32 * lg + 30 : 32 * lg + 32, :] += np.float16(120.0)
    iota_d = nc.inline_tensor(iota_np, name="iota60")

    with tile.TileContext(nc) as tc, tc.tile_pool(name="const", bufs=1) as cpool, \
            tc.tile_pool(name="ctrl", bufs=1) as ctrl_pool, \
            tc.tile_pool(name="outp", bufs=1) as out_pool, \
            tc.tile_pool(name="stage", bufs=4) as stage_pool, \
            tc.tile_pool(name="band", bufs=3) as band_pool, \
            tc.tile_pool(name="rpsum", bufs=3, space="PSUM") as rps_pool, \
            tc.tile_pool(name="imgpsum", bufs=2, space="PSUM") as img_pool:

        # Prologue: DMA loads land in staging tiles; DVE copies them into the
        # tiles PE reads (PE LDWEIGHTS tolerates very few sync waits).
        tsc0 = cpool.tile([4, 32], f32, tag="tsc0")
        nc.sync.dma_start(tsc0[:], tsc_d.ap())
        tsc = cpool.tile([4, 32], f32, tag="tsc")
        nc.vector.tensor_copy(tsc[:], tsc0[:])
        iot = cpool.tile([PTS, CW], f16, tag="iota")
        nc.sync.dma_start(iot[:], iota_d.ap())

        # control points: partition k (4), free = (b, l, coord).  All group
        # DMAs issue up front (parallel queues); the DVE copy for group g is
        # emitted lazily before the first sample that needs it.
        GRP = 8
        NGRP = BC // GRP
        ct = ctrl_pool.tile([4, BC * 2 * L], f32, tag="ct")
        gsz = GRP * 2 * L
        ct_stage = []
        for g in range(NGRP):
            ct0 = stage_pool.tile([4, gsz], f32, tag="ct0")
            nc.sync.dma_start(
                ct0[:].rearrange("k (b l c) -> k b l c", b=GRP, c=2),
                x_t.ap()[g * GRP : (g + 1) * GRP]
                .rearrange("b l (k c) -> k b l c", k=4),
            )
            ct_stage.append(ct0)
        ct_v = ct[:].rearrange("k (b c g co) -> k b c g co", b=BC, c=CHUNKS, co=2)

        # all 32 output images live here until the per-group DMAs
        out_all = out_pool.tile([W, BC * W], f32, tag="oall")

        CS_ALL = 2 * CHUNKS
        groups_emitted = [False] * NGRP
        r_tiles = {}
        img_tiles = {}

        def emit_r(b):
            g = b // GRP
            if not groups_emitted[g]:
                groups_emitted[g] = True
                nc.vector.tensor_copy(
                    ct[:, g * gsz : (g + 1) * gsz], ct_stage[g][:]
                )
            r_ps = rps_pool.tile([PTS, 2 * CHUNKS], f32, tag="rps")
            for lg in range(4):
                nc.tensor.matmul(
                    r_ps[32 * lg : 32 * lg + 32, :],
                    lhsT=tsc[:],
                    rhs=ct_v[:, b : b + 1, :, lg : lg + 1, :],
                    start=True,
                    stop=True,
                    tile_position=(0, 32 * lg),
                )
            r_tiles[b] = r_ps

        def emit_min(b):
            """min(scale*img, 1) -> staging; group DMA when a group closes."""
            img = img_tiles.pop(b)
            nc.vector.tensor_scalar(
                out_all[:, W * b : W * (b + 1)],
                img[:],
                DERF_FIX,
                1.0,
                mybir.AluOpType.mult,
                mybir.AluOpType.min,
            )
            if b % GRP == GRP - 1:
                g = b // GRP
                nc.sync.dma_start(
                    y_t.ap()[g * GRP : (g + 1) * GRP].rearrange("b i j -> i b j"),
                    out_all[:, W * GRP * g : W * GRP * (g + 1)]
                    .rearrange("i (b j) -> i b j", b=GRP),
                )

        # software pipeline: r two samples ahead, min one sample behind.
        emit_r(0)
        emit_r(1)

        for b in range(BC):
            if b + 2 < BC:
                emit_r(b + 2)
            r_ps = r_tiles.pop(b)

            # ---- banded distance, fp16; r read straight from PSUM ----
            dd = band_pool.tile([PTS, 2 * CHUNKS * CW], f16, tag="dd")
            nc.vector.tensor_tensor(
                dd[:].rearrange("p (cs w) -> p cs w", w=CW),
                iot[:].rearrange("p (o w) -> p o w", o=1).broadcast_to(
                    [PTS, CS_ALL, CW]
                ),
                r_ps[:].rearrange("p (cs o) -> p cs o", o=1).broadcast_to(
                    [PTS, CS_ALL, CW]
                ),
                mybir.AluOpType.subtract,
            )

            gg = band_pool.tile([PTS, 2 * CHUNKS * CW], f16, tag="gg")
            nc.scalar.activation(
                gg[:], dd[:],
                mybir.ActivationFunctionType.Derivative_Erf,
                bias=0.0, scale=SDERF,
            )

            # ---- image accumulation: sum_c GxT_c^T @ GyT_c ----
            img = img_pool.tile([W, W], f32, tag="img")
            for c in range(CHUNKS):
                nc.tensor.matmul(
                    img[:],
                    lhsT=gg[:, 2 * CW * c : 2 * CW * c + W],
                    rhs=gg[:, 2 * CW * c + CW : 2 * CW * c + CW + W],
                    start=(c == 0),
                    stop=(c == CHUNKS - 1),
                )
            img_tiles[b] = img

            if b > 0:
                emit_min(b - 1)
        emit_min(BC - 1)

    if legalize:
        _legalize_waits(nc)
    return nc


_PROGRAM = None


def kernel(x: np.ndarray, _trace: bool = False) -> np.ndarray:
    global _PROGRAM, LAST_RESULTS
    assert x.shape == (B, L, 8) and x.dtype == np.float32, (x.shape, x.dtype)
    if _PROGRAM is None:
        _PROGRAM = build_program()
    nc = _PROGRAM
    shards = np.split(np.ascontiguousarray(x), NCORES, axis=0)
    in_maps = [{"x": s} for s in shards]
    res = run_bass_kernel_spmd(nc, in_maps, list(range(NCORES)), trace=_trace)
    LAST_RESULTS = res
    return np.concatenate([res.results[i]["y"] for i in range(NCORES)], axis=0)
